# revision 12
# baseline (speedup 1.0000x reference)
"""Trainium2 Bass kernel for MF embedding-lookup + dot-product scoring.

out[u, i] = dot(user_hiddens[user_ids[u]], item_hiddens[item_ids[i]])

Sharding: user-parallel over 8 cores - each core owns 512 users x all 4096
items; tables replicated in every core's HBM. Per core:
  - users: 4 generic indirect-DMA gathers (one per 128-user tile; the HW
    SWDGE ucode supports one index per partition per call)
  - items: 8 custom dma_gather calls over static 12500-row windows of the
    item table (int16 indices = id - window_base, trailing -1 padding to a
    640 capacity per window - the SWDGE descriptor ring tops out between
    640 and 1024 indices per call; host bucket-sorts ids and unpermutes
    output columns afterwards)
  - user and item gathers interleaved so matmuls and output DMA start
    while later windows still gather
  - PE transpose to [64, batch] staged up to 4 tiles per PSUM bank
  - plain bf16 matmuls (tolerance 2e-2; bf16 on positive uniform data
    gives ~4e-3), N=512 f32 PSUM blocks, copies cast to bf16
  - output written as bf16 [512, 5120] per core in thirds per user tile;
    host drops pad columns, unpermutes, casts back to f32
  - warm-up matmuls keep the PE out of its low p-state
"""

import numpy as np

import concourse.bacc as bacc
import concourse.bass as bass
import concourse.mybir as mybir
import concourse.tile as tile
from concourse import library_config
from concourse.bass_utils import run_bass_kernel_spmd
from concourse.masks import make_identity

NUM_USERS = 1_000_000
NUM_ITEMS = 100_000
D = 64
BU = 4096
BI = 4096
N_CORES = 8
UC = BU // N_CORES  # users per core = 512
P = 128
UT = UC // P        # user tiles per core = 4
NWIN = 8            # item windows
WBASE = 12_500      # window stride
CAP = 640           # padded item capacity per window (5 tiles of 128)
WT = CAP // P       # item tiles per window = 5
ICAP = NWIN * CAP   # padded item count = 5120
NBLK = 512          # matmul moving free dim
NB = ICAP // NBLK   # item blocks = 10
WCOL = CAP // 16    # iidx cols per window = 40

# bucket w unlocks these matmul blocks (block n needs cols [512n, 512n+512)
# which span buckets (512n)//640 .. (512n+511)//640)
_BLOCKS_AT = {0: [0], 1: [1], 2: [2], 3: [3, 4], 4: [5], 5: [6], 6: [7], 7: [8, 9]}
_FLUSH = {3: (0, 2048), 6: (2048, 3584), 9: (3584, 5120)}

_cache = {}


def _build():
    nc = bacc.Bacc()
    ut_dram = nc.dram_tensor(
        "user_table", [NUM_USERS, D], mybir.dt.float32, kind="ExternalInput"
    )
    it_dram = nc.dram_tensor(
        "item_table", [NUM_ITEMS, D], mybir.dt.float32, kind="ExternalInput"
    )
    uid_dram = nc.dram_tensor("uids", [P, UT], mybir.dt.int32, kind="ExternalInput")
    iidx_dram = nc.dram_tensor(
        "iidx", [P, NWIN * WCOL], mybir.dt.int16, kind="ExternalInput"
    )
    out_dram = nc.dram_tensor(
        "out", [UC, ICAP], mybir.dt.bfloat16, kind="ExternalOutput"
    )

    f32 = mybir.dt.float32
    bf16 = mybir.dt.bfloat16

    with tile.TileContext(nc) as tc:
        with (
            tc.tile_pool(name="const", bufs=1) as constp,
            tc.tile_pool(name="idx", bufs=1) as idxp,
            tc.tile_pool(name="gath", bufs=1) as gathp,
            tc.tile_pool(name="stk", bufs=1) as stkp,
            tc.tile_pool(name="tp", bufs=2, space="PSUM") as tpp,
            tc.tile_pool(name="mm", bufs=2, space="PSUM") as mmp,
            tc.tile_pool(name="outp", bufs=4) as outp,
        ):
            ident = constp.tile([P, P], f32)
            make_identity(nc, ident[:])

            uids = idxp.tile([P, UT], mybir.dt.int32)
            iidx = idxp.tile([P, NWIN * WCOL], mybir.dt.int16)
            nc.sync.dma_start(out=uids[:], in_=uid_dram[:])
            nc.sync.dma_start(out=iidx[:], in_=iidx_dram[:])

            # PE warm-up to escape the low p-state during the gather phase
            for w in range(6):
                junk = mmp.tile([P, NBLK], f32)
                nc.tensor.matmul(
                    junk[:, 0:P], lhsT=ident[:], rhs=ident[:],
                    start=True, stop=True, skip_group_check=True,
                )

            nc.gpsimd.load_library(library_config.mlp)

            gu = [gathp.tile([P, D], f32, name=f"gu{k}") for k in range(UT)]
            gi = [gathp.tile([P, WT, D], f32, name=f"gi{k}") for k in range(NWIN)]

            def gather_user(k):
                nc.gpsimd.indirect_dma_start(
                    out=gu[k][:],
                    out_offset=None,
                    in_=ut_dram[:],
                    in_offset=bass.IndirectOffsetOnAxis(
                        ap=uids[:, k : k + 1], axis=0
                    ),
                )

            def gather_items(w):
                lo = w * WBASE
                hi = min(lo + 32_000, NUM_ITEMS)
                nc.gpsimd.dma_gather(
                    gi[w][:],
                    it_dram[lo:hi, :],
                    iidx[:, w * WCOL : (w + 1) * WCOL],
                    CAP, CAP, D,
                )

            ustack = stkp.tile([D, UC], bf16)    # [64, 512]
            vstack = stkp.tile([D, ICAP], bf16)  # [64, 5120]

            cp_rot = [0]

            def copy(dst, src):
                e = cp_rot[0] % 2
                cp_rot[0] += 1
                if e == 0:
                    nc.scalar.copy(out=dst, in_=src)
                else:
                    nc.vector.tensor_copy(out=dst, in_=src)

            def u_transpose(t):
                ps = tpp.tile([D, 4 * P], f32)
                nc.tensor.transpose(ps[:, 0:P], gu[t][:], ident[:])
                copy(ustack[:, t * P : (t + 1) * P], ps[:, 0:P])

            def item_transposes(w):
                # 5 tiles: one PSUM bank of 4 + one of 1, 2 copies
                base = w * CAP
                for grp, n in ((0, 4), (1, 1)):
                    ps = tpp.tile([D, 4 * P], f32)
                    for j in range(n):
                        c = grp * 4 + j
                        nc.tensor.transpose(
                            ps[:, j * P : (j + 1) * P], gi[w][:, c, :], ident[:]
                        )
                    sl = slice(base + grp * 4 * P, base + grp * 4 * P + n * P)
                    copy(vstack[:, sl], ps[:, 0 : n * P])

            ot = [outp.tile([P, ICAP], bf16, name=f"ot{k}") for k in range(UT)]

            def mm(t, n):
                po = mmp.tile([P, NBLK], f32)
                nc.tensor.matmul(
                    po[:],
                    lhsT=ustack[:, t * P : (t + 1) * P],
                    rhs=vstack[:, n * NBLK : (n + 1) * NBLK],
                    start=True,
                    stop=True,
                )
                copy(ot[t][:, n * NBLK : (n + 1) * NBLK], po[:])
                if n in _FLUSH:
                    a, b = _FLUSH[n]
                    nc.sync.dma_start(
                        out=out_dram[t * P : (t + 1) * P, a:b],
                        in_=ot[t][:, a:b],
                    )

            # --- interleaved schedule ---
            # gathers: u0 i0 i1 u1 i2 i3 u2 i4 i5 u3 i6 i7
            # compute follows arrival; emit all (t, n) pairs once both the
            # user tile t and every bucket of block n have been emitted.
            n_users = [0]   # user tiles gathered so far
            n_blocks = [0]  # matmul blocks unlocked so far
            done = set()

            def emit_ready():
                for t in range(n_users[0]):
                    for n in range(n_blocks[0]):
                        if (t, n) not in done:
                            done.add((t, n))
                            mm(t, n)

            gather_seq = ["u0", "i0", "i1", "u1", "i2", "i3", "u2", "i4",
                          "i5", "u3", "i6", "i7"]
            for g in gather_seq:
                kind, k = g[0], int(g[1:])
                if kind == "u":
                    gather_user(k)
                else:
                    gather_items(k)
            # compute emission mirrors arrival order
            for g in gather_seq:
                kind, k = g[0], int(g[1:])
                if kind == "u":
                    u_transpose(k)
                    n_users[0] = k + 1
                else:
                    item_transposes(k)
                    n_blocks[0] = max(n_blocks[0], 1 + max(_BLOCKS_AT[k]))
                emit_ready()

    nc.finalize()
    return nc


def kernel(user_hiddens, item_hiddens, user_ids, item_ids, **_):
    user_hiddens = np.ascontiguousarray(user_hiddens, dtype=np.float32)
    item_hiddens = np.ascontiguousarray(item_hiddens, dtype=np.float32)
    user_ids = np.asarray(user_ids)
    item_ids = np.asarray(item_ids).astype(np.int64)

    if "nc" not in _cache:
        _cache["nc"] = _build()
    nc = _cache["nc"]

    # bucket items by window (shared by all cores)
    win = item_ids // WBASE
    order = np.argsort(win, kind="stable")
    counts = np.bincount(win, minlength=NWIN)
    assert counts.max() <= CAP, f"item window overflow: {counts}"
    iidx = np.empty((P, NWIN * WCOL), dtype=np.int16)
    for w in range(NWIN):
        ids_w = item_ids[order[counts[:w].sum() : counts[: w + 1].sum()]]
        # pad with a valid in-window id (0 = window base row), NOT -1: the
        # Q7 ucode trims trailing negatives, pushing fewer ring descriptors
        # than the decode side reserved, which corrupts later SWDGE calls.
        arr = np.zeros(CAP, dtype=np.int16)
        arr[: counts[w]] = (ids_w - w * WBASE).astype(np.int16)
        wrapped = arr.reshape(WCOL, 16).T        # [16, 40]
        iidx[:, w * WCOL : (w + 1) * WCOL] = np.tile(wrapped, (8, 1))

    in_maps = []
    for c in range(N_CORES):
        uc = user_ids[c * UC : (c + 1) * UC]
        uids_t = np.ascontiguousarray(uc.astype(np.int32).reshape(UT, P).T)
        in_maps.append(
            {
                "user_table": user_hiddens,
                "item_table": item_hiddens,
                "uids": uids_t,
                "iidx": iidx,
            }
        )

    res = run_bass_kernel_spmd(nc, in_maps, list(range(N_CORES)))
    full = np.concatenate(
        [np.asarray(res.results[c]["out"]) for c in range(N_CORES)], axis=0
    ).astype(np.float32)  # [4096, 5120]
    valid = np.concatenate(
        [full[:, w * CAP : w * CAP + counts[w]] for w in range(NWIN)], axis=1
    )  # [4096, 4096] in bucketed order
    out = np.empty((BU, BI), dtype=np.float32)
    out[:, order] = valid
    return out


# revision 14
# speedup vs baseline: 1.7447x; 1.7447x over previous
"""Trainium2 Bass kernel for MF embedding-lookup + dot-product scoring.

out[u, i] = dot(user_hiddens[user_ids[u]], item_hiddens[item_ids[i]])

Sharding: 2D over 8 cores - 4 user groups (1024 users) x 2 item groups
(2048 items); tables replicated in every core's HBM.

The hard constraint on TRN2 is SWDGE descriptor generation: every
indirect-DMA call costs ~1us fixed + ~2.5ns/row on the single GpSimd
queue, one 128-row call per user/item tile (the HW ucode supports one
index per partition per call; the custom dma_gather ucode is ~10ns/idx -
slower). 24 calls/core is the minimum for a rectangular sharding, so the
kernel hides everything else behind that ~28us serialized gather stream:
  - user and item gathers interleaved (u,i,i x 8) so transposes, matmuls
    and output DMA start after the third call
  - PE transpose to [64, batch]; psum staged 2 item tiles per bank
  - plain bf16 matmuls (tolerance 2e-2; bf16 on positive uniform data
    gives ~4e-3), N=512 f32 PSUM blocks, copies cast to bf16 alternating
    scalar/vector
  - output written as bf16 [1024, 2048] per core, flushed in halves per
    user tile; host casts back to f32
  - warm-up matmuls keep the PE out of its low p-state
"""

import numpy as np

import concourse.bacc as bacc
import concourse.bass as bass
import concourse.mybir as mybir
import concourse.tile as tile
from concourse.bass_utils import run_bass_kernel_spmd
from concourse.masks import make_identity

NUM_USERS = 1_000_000
NUM_ITEMS = 100_000
D = 64
BU = 4096
BI = 4096
N_CORES = 8
RU = 4              # user groups
RI = 2              # item groups
UC = BU // RU       # users per core = 1024
IC = BI // RI       # items per core = 2048
P = 128
UT = UC // P        # user tiles per core = 8
IT = IC // P        # item tiles per core = 16
NBLK = 512          # matmul moving free dim
NB = IC // NBLK     # item blocks = 4

_cache = {}


def _build():
    nc = bacc.Bacc()
    ut_dram = nc.dram_tensor(
        "user_table", [NUM_USERS, D], mybir.dt.float32, kind="ExternalInput"
    )
    it_dram = nc.dram_tensor(
        "item_table", [NUM_ITEMS, D], mybir.dt.float32, kind="ExternalInput"
    )
    # ids[p, 0:8] = user tile ids, ids[p, 8:24] = item tile ids
    ids_dram = nc.dram_tensor(
        "ids", [P, UT + IT], mybir.dt.int32, kind="ExternalInput"
    )
    out_dram = nc.dram_tensor(
        "out", [UC, IC], mybir.dt.bfloat16, kind="ExternalOutput"
    )

    f32 = mybir.dt.float32
    bf16 = mybir.dt.bfloat16

    with tile.TileContext(nc) as tc:
        with (
            tc.tile_pool(name="const", bufs=1) as constp,
            tc.tile_pool(name="idx", bufs=1) as idxp,
            tc.tile_pool(name="gath", bufs=1) as gathp,
            tc.tile_pool(name="stk", bufs=1) as stkp,
            tc.tile_pool(name="tp", bufs=2, space="PSUM") as tpp,
            tc.tile_pool(name="mm", bufs=2, space="PSUM") as mmp,
            tc.tile_pool(name="outp", bufs=1) as outp,
        ):
            ident = constp.tile([P, P], f32)
            make_identity(nc, ident[:])

            ids = idxp.tile([P, UT + IT], mybir.dt.int32)
            nc.sync.dma_start(out=ids[:], in_=ids_dram[:])

            # PE warm-up to hold p-state through the gather phase
            for w in range(6):
                junk = mmp.tile([P, NBLK], f32)
                nc.tensor.matmul(
                    junk[:, 0:P], lhsT=ident[:], rhs=ident[:],
                    start=True, stop=True, skip_group_check=True,
                )

            gu = [gathp.tile([P, D], f32, name=f"gu{k}") for k in range(UT)]
            gv = [gathp.tile([P, D], f32, name=f"gv{k}") for k in range(IT)]

            def gather(dst, table, col):
                nc.gpsimd.indirect_dma_start(
                    out=dst[:],
                    out_offset=None,
                    in_=table[:],
                    in_offset=bass.IndirectOffsetOnAxis(
                        ap=ids[:, col : col + 1], axis=0
                    ),
                )

            ustack = stkp.tile([D, UC], bf16)   # [64, 1024]
            vstack = stkp.tile([D, IC], bf16)   # [64, 2048]

            cp_rot = [0]

            def copy(dst, src):
                e = cp_rot[0] % 2
                cp_rot[0] += 1
                if e == 0:
                    nc.scalar.copy(out=dst, in_=src)
                else:
                    nc.vector.tensor_copy(out=dst, in_=src)

            def u_transpose(t):
                ps = tpp.tile([D, NBLK], f32)
                nc.tensor.transpose(ps[:, 0:P], gu[t][:], ident[:])
                copy(ustack[:, t * P : (t + 1) * P], ps[:, 0:P])

            def i_transpose_pair(j):
                # item tiles 2j, 2j+1 -> one psum tile, one copy
                ps = tpp.tile([D, NBLK], f32)
                for q in range(2):
                    nc.tensor.transpose(
                        ps[:, q * P : (q + 1) * P], gv[2 * j + q][:], ident[:]
                    )
                copy(vstack[:, 2 * j * P : (2 * j + 2) * P], ps[:, 0 : 2 * P])

            ot = [outp.tile([P, IC], bf16, name=f"ot{k}") for k in range(UT)]

            def mm(t, n):
                po = mmp.tile([P, NBLK], f32)
                nc.tensor.matmul(
                    po[:],
                    lhsT=ustack[:, t * P : (t + 1) * P],
                    rhs=vstack[:, n * NBLK : (n + 1) * NBLK],
                    start=True,
                    stop=True,
                )
                copy(ot[t][:, n * NBLK : (n + 1) * NBLK], po[:])
                if n % 2 == 1:  # flush half rows [0:1024) / [1024:2048)
                    h = n // 2
                    sl = slice(h * 2 * NBLK, (h + 1) * 2 * NBLK)
                    nc.sync.dma_start(
                        out=out_dram[t * P : (t + 1) * P, sl],
                        in_=ot[t][:, sl],
                    )

            # --- interleaved schedule: u,i,i repeated 8x ---
            for k in range(UT):
                gather(gu[k], ut_dram, k)
                gather(gv[2 * k], it_dram, UT + 2 * k)
                gather(gv[2 * k + 1], it_dram, UT + 2 * k + 1)

            done = set()

            def emit_ready(users_done, items_done):
                blocks = min(NB, items_done // 4)
                for t in range(users_done):
                    for n in range(blocks):
                        if (t, n) not in done:
                            done.add((t, n))
                            mm(t, n)

            for k in range(UT):
                u_transpose(k)
                i_transpose_pair(k)
                emit_ready(k + 1, 2 * k + 2)
            # all gathers emitted; drain remaining blocks
            emit_ready(UT, IT)

    nc.finalize()
    return nc


def kernel(user_hiddens, item_hiddens, user_ids, item_ids, **_):
    user_hiddens = np.ascontiguousarray(user_hiddens, dtype=np.float32)
    item_hiddens = np.ascontiguousarray(item_hiddens, dtype=np.float32)
    user_ids = np.asarray(user_ids)
    item_ids = np.asarray(item_ids)

    if "nc" not in _cache:
        _cache["nc"] = _build()
    nc = _cache["nc"]

    in_maps = []
    for c in range(N_CORES):
        cu, ci = divmod(c, RI)
        uc = user_ids[cu * UC : (cu + 1) * UC]
        icd = item_ids[ci * IC : (ci + 1) * IC]
        ids_t = np.empty((P, UT + IT), dtype=np.int32)
        ids_t[:, :UT] = uc.astype(np.int32).reshape(UT, P).T
        ids_t[:, UT:] = icd.astype(np.int32).reshape(IT, P).T
        in_maps.append(
            {
                "user_table": user_hiddens,
                "item_table": item_hiddens,
                "ids": np.ascontiguousarray(ids_t),
            }
        )

    res = run_bass_kernel_spmd(nc, in_maps, list(range(N_CORES)))
    out = np.empty((BU, BI), dtype=np.float32)
    for c in range(N_CORES):
        cu, ci = divmod(c, RI)
        out[cu * UC : (cu + 1) * UC, ci * IC : (ci + 1) * IC] = np.asarray(
            res.results[c]["out"]
        ).astype(np.float32)
    return out
